# revision 37
# baseline (speedup 1.0000x reference)
"""Trainium2 Bass kernel for nn_LocalSmoother (LN -> QKV -> RoPE -> 32-token
block-diagonal attention -> out-proj -> residual).

Sharding: B*L = 16384 tokens split evenly across 8 cores (2048 tokens each,
64 chunks of 32). Attention is block-diagonal over 32-token chunks, so shards
are fully independent (pure SPMD, no collectives). Weights are replicated.

v2 highlights over the original implementation:
  - QKV and out-proj run in fp8e4 (e4m3) with MatmulPerfMode.DoubleRow:
    2-deep k-pairs at 0.5 cycles/row -> 4x the fp16 PE throughput.
  - LayerNorm emits xn directly in fp8 (scaled); the feature transpose moves
    fp8 byte-PAIRS through the fp16 DMA xbar (half the transpose traffic).
    The resulting pair-interleaved layout is exactly DoubleRow's k-pair shape.
  - The 32-token block-diagonal softmax mask is pre-seeded into the scores
    PSUM with one 5-row constant matmul (rank-5 decomposition of the mask
    bias), replacing the per-headgroup gpsimd mask multiply.
  - LN stats via bn_stats/bn_aggr (one DVE pass instead of two reductions).
  - Residual add on gpsimd, xn/v-evict/asb-evict on Act, to keep DVE free
    for the ops only it can do fast.

Scaling ledger (all folded, no standalone dequant ops):
    xn8 = xn * SX                     (LN activation scale)
    w8  = W^T * SW                    (host)
    q/k = psum * (cos/(SX*SW))        (cos table pre-scaled)
    v16 = psum * (SA/(SX*SW))         (v-evict activation scale)
    asb = psum(PV of normalized P)    (already SA-scaled via v16)
    wo8 = Wo^T * SW2                  (host)
    y   = psum * 1/(SA*SW2) + x       (residual scalar_tensor_tensor)
"""

import sys
import numpy as np
from contextlib import ExitStack

sys.path.insert(0, "/opt/trn_rl_repo")

D_MODEL = 1024
N_HEADS = 16
D_HEAD = 64
CHUNK = 32
LN_EPS = 1e-5
ROPE_BASE = 10000.0

N_CORES = 8
BLK = 512          # tokens per pipeline block
SUB = 128          # tokens per partition tile
NSUB = BLK // SUB  # 4
ND = D_MODEL // 128  # 8 feature tiles
NG = D_MODEL // 256  # 4 feature pair-groups

SX = 8.0     # xn fp8 scale
SW = 32.0    # qkv weight fp8 scale
SA = 16.0    # attention-output fp8 scale (folded into v16)
SW2 = 32.0   # out-proj weight fp8 scale
MASKB = 30000.0


def build_program(T, with_beta=False, stop_stage=None, repeats=1):
    """Build the per-core Bass program for a T-token shard."""
    import concourse.bass as bass
    import concourse.tile as tile
    from concourse import bacc, mybir

    dt = mybir.dt
    AF = mybir.ActivationFunctionType
    OP = mybir.AluOpType
    DR = mybir.MatmulPerfMode.DoubleRow

    NBLK = T // BLK
    nc = bacc.Bacc("TRN2", target_bir_lowering=False, debug=False,
                   num_devices=N_CORES)

    xs = nc.dram_tensor("xs", [T, D_MODEL], dt.float32, kind="ExternalInput").ap()
    # fp8 weights, k-plane pairs along dim1 for DoubleRow
    wqk = nc.dram_tensor("wqk", [128, ND, 2 * D_MODEL], dt.float8e4, kind="ExternalInput").ap()
    wv = nc.dram_tensor("wv", [128, ND, D_MODEL], dt.float8e4, kind="ExternalInput").ap()
    wo = nc.dram_tensor("wo", [128, ND, D_MODEL], dt.float8e4, kind="ExternalInput").ap()
    cosb = nc.dram_tensor("cosb", [128, CHUNK], dt.float16, kind="ExternalInput").ap()
    tanb = nc.dram_tensor("tanb", [128, CHUNK], dt.float16, kind="ExternalInput").ap()
    c5 = nc.dram_tensor("c5", [128, 128], dt.bfloat16, kind="ExternalInput").ap()
    brow5 = nc.dram_tensor("brow5", [128, BLK], dt.bfloat16, kind="ExternalInput").ap()
    kb = nc.dram_tensor("kb", [128, T // 128], dt.float32, kind="ExternalInput").ap()
    beta = None
    if with_beta:
        beta = nc.dram_tensor("beta", [128, D_MODEL], dt.float16, kind="ExternalInput").ap()
    ys = nc.dram_tensor("ys", [T, D_MODEL], dt.float32, kind="ExternalOutput").ap()

    with tile.TileContext(nc) as tc, ExitStack() as ctx:
        const = ctx.enter_context(tc.tile_pool(name="const", bufs=1))
        # ---- constants ----
        wqk_sb = const.tile([128, ND, 2 * D_MODEL], dt.float8e4, tag="wqk")
        nc.sync.dma_start(wqk_sb[:], wqk)
        wv_sb = const.tile([128, ND, D_MODEL], dt.float8e4, tag="wv")
        nc.sync.dma_start(wv_sb[:], wv)
        wo_sb = const.tile([128, ND, D_MODEL], dt.float8e4, tag="wo")
        nc.sync.dma_start(wo_sb[:], wo)
        cos_sb = const.tile([128, CHUNK], dt.float16, tag="cos")
        nc.sync.dma_start(cos_sb[:], cosb)
        tan_sb = const.tile([128, CHUNK], dt.float16, tag="tan")
        nc.sync.dma_start(tan_sb[:], tanb)
        c5_sb = const.tile([128, 128], dt.bfloat16, tag="c5")
        nc.sync.dma_start(c5_sb[:], c5)
        brow5_sb = const.tile([128, BLK], dt.bfloat16, tag="brow5")
        nc.sync.dma_start(brow5_sb[:], brow5)
        kb_sb = const.tile([128, T // 128], dt.float32, tag="kb")
        nc.sync.dma_start(kb_sb[:], kb)
        ones_sb = const.tile([128, 1], dt.float16, tag="ones")
        nc.gpsimd.memset(ones_sb[:], 1.0)
        ones64_sb = const.tile([128, 64], dt.float16, tag="ones64")
        nc.gpsimd.memset(ones64_sb[:], 1.0)
        eps_sb = const.tile([128, 1], dt.float32, tag="eps")
        nc.gpsimd.memset(eps_sb[:], LN_EPS / (SX * SX))
        beta_sb = None
        if with_beta:
            beta_sb = const.tile([128, D_MODEL], dt.float16, tag="beta")
            nc.sync.dma_start(beta_sb[:], beta)

        cos_bc = cos_sb[:].unsqueeze(1).to_broadcast((128, BLK // CHUNK, CHUNK))
        tan_bc_big = tan_sb[:].unsqueeze(1).to_broadcast(
            (128, ND * BLK // CHUNK, CHUNK))

        # ---- pools ----
        xp = ctx.enter_context(tc.tile_pool(name="xp", bufs=8))
        stp = ctx.enter_context(tc.tile_pool(name="stp", bufs=6))
        xnp = ctx.enter_context(tc.tile_pool(name="xnp", bufs=6))
        xtp = ctx.enter_context(tc.tile_pool(name="xtp", bufs=2))
        qcp = ctx.enter_context(tc.tile_pool(name="qcp", bufs=2))
        qsp = ctx.enter_context(tc.tile_pool(name="qsp", bufs=2))
        vp = ctx.enter_context(tc.tile_pool(name="vp", bufs=6))
        pep = ctx.enter_context(tc.tile_pool(name="pep", bufs=5))
        rcp = ctx.enter_context(tc.tile_pool(name="rcp", bufs=2))
        asp = ctx.enter_context(tc.tile_pool(name="asp", bufs=3))
        yp = ctx.enter_context(tc.tile_pool(name="yp", bufs=3))

        psA = ctx.enter_context(tc.tile_pool(name="psA", bufs=3, space="PSUM"))
        psS = ctx.enter_context(tc.tile_pool(name="psS", bufs=1, space="PSUM"))
        psB = ctx.enter_context(tc.tile_pool(name="psB", bufs=2, space="PSUM"))

        inv_sx2 = 1.0 / (SX * SX)

        # one persistent sums bank: 4 headgroups stack at partition bases
        # 0/32/64/96 so a single reciprocal covers a whole 128-token tile
        sums_bank = psS.tile([128, BLK], dt.float32, tag="sum")
        nc.vector.memset(sums_bank[:], 1.0)

        def load_ln_stage(b):
            """Load x, LayerNorm, transpose, fp8-convert for block b.
            Emitted one block ahead of the consuming stages (software
            pipeline) so the load/transpose DMA queues stay ahead."""
            t0 = b * BLK
            xn_tiles = []
            x_tiles = []
            for tt in range(NSUB):
                xt = xp.tile([128, D_MODEL], dt.float32, tag="x")
                x_tiles.append(xt)
                nc.sync.dma_start(xt[:], xs[t0 + tt * SUB: t0 + (tt + 1) * SUB, :])
                st6 = stp.tile([128, 2, 6], dt.float32, tag="st6")
                nc.vector.bn_stats(st6[:, 0], xt[:, 0:512])
                nc.vector.bn_stats(st6[:, 1], xt[:, 512:1024])
                mv = stp.tile([128, 2], dt.float32, tag="mv")
                nc.vector.bn_aggr(mv[:], st6[:])
                # std8 = sqrt((var+eps))/SX ; rstd_s = SX/std
                std8 = stp.tile([128, 1], dt.float32, tag="sd")
                nc.scalar.activation(std8[:], mv[:, 1:2], AF.Sqrt,
                                     scale=inv_sx2, bias=eps_sb[:])
                rstd_s = stp.tile([128, 1], dt.float32, tag="rs")
                nc.vector.reciprocal(rstd_s[:], std8[:])
                nbias = stp.tile([128, 1], dt.float32, tag="nb")
                nc.vector.scalar_tensor_tensor(nbias[:], mv[:, 0:1], -1.0,
                                               rstd_s[:], op0=OP.mult,
                                               op1=OP.mult)
                xn = xnp.tile([128, D_MODEL], dt.float16, tag="xn")
                nc.scalar.activation(xn[:], xt[:], AF.Identity,
                                     scale=rstd_s[:], bias=nbias[:])
                if with_beta:
                    # beta scaled by SX on host
                    nc.vector.tensor_tensor(xn[:], xn[:], beta_sb[:], op=OP.add)
                xn_tiles.append(xn)

            XT16 = xtp.tile([128, ND, BLK], dt.float16, tag="xt16")
            for tt in range(NSUB):
                for dtile in range(ND):
                    eng = nc.sync if (tt * ND + dtile) % 2 == 0 else nc.scalar
                    eng.dma_start_transpose(
                        XT16[:, dtile, tt * SUB:(tt + 1) * SUB],
                        xn_tiles[tt][:, dtile * 128:(dtile + 1) * 128])
            return x_tiles, xn_tiles, XT16

        def convert_stage(XT16):
            # gpsimd does the fp16->fp8 cast: DVE's cast rounds badly.
            # Emitted after the previous block's rope mults so the Pool
            # queue's critical-path work is not stuck behind it.
            XT = xtp.tile([128, ND, BLK], dt.float8e4, tag="xt")
            for g in range(NG):
                nc.gpsimd.tensor_copy(XT[:, 2 * g:2 * g + 2, :],
                                      XT16[:, 2 * g:2 * g + 2, :])
            return XT

        staged = load_ln_stage(0)
        staged_xt = convert_stage(staged[2])
        for it in range(NBLK * repeats):
            b = it % NBLK
            t0 = b * BLK
            x_tiles, xn_tiles, XT16 = staged
            XT = staged_xt
            if it + 1 < NBLK * repeats:
                staged = load_ln_stage((it + 1) % NBLK)

            if stop_stage == 'ln':
                dbg = yp.tile([128, D_MODEL], dt.float32, tag="y")
                nc.vector.tensor_copy(dbg[:], xn_tiles[0][:])
                nc.sync.dma_start(ys[t0:t0 + SUB, :], dbg[:])
                continue
            if stop_stage == 'xt':
                dbg = yp.tile([128, D_MODEL], dt.float32, tag="y")
                nc.vector.tensor_copy(dbg[:, 0:512], XT16[:, 0, :])
                nc.sync.dma_start(ys[t0:t0 + SUB, :], dbg[:])
                continue

            # ---------- qk projection (fp8 DoubleRow) + cos fuse ----------
            q_all = qcp.tile([128, ND, BLK], dt.float16, tag="qall")
            k_all = qcp.tile([128, ND, BLK], dt.float16, tag="kall")
            for et in range(16):
                ps = psA.tile([128, BLK], dt.float32, tag="ps512")
                for g in range(NG):
                    nc.tensor.matmul(ps[:],
                                     wqk_sb[:, 2 * g:2 * g + 2, et * 128:(et + 1) * 128],
                                     XT[:, 2 * g:2 * g + 2, :],
                                     start=(g == 0), stop=(g == NG - 1),
                                     perf_mode=DR)
                tgt = q_all if et < 8 else k_all
                nc.vector.tensor_tensor(
                    tgt[:, et % 8, :].rearrange("p (a c) -> p a c", c=CHUNK),
                    ps[:].rearrange("p (a c) -> p a c", c=CHUNK),
                    cos_bc, op=OP.mult)

            if stop_stage == 'qk':
                dbg = yp.tile([128, D_MODEL], dt.float32, tag="y")
                nc.vector.tensor_copy(dbg[:, 0:512], q_all[:, 0, :])
                nc.sync.dma_start(ys[t0:t0 + SUB, :], dbg[:])
                continue

            # ---------- v projection (fp8 DoubleRow, token-partition) ------
            v_tiles = []
            for tt in range(NSUB):
                vt = vp.tile([128, D_MODEL], dt.float16, tag="v")
                pv2 = psB.tile([128, D_MODEL], dt.float32, tag="big")
                for n in range(2):
                    for g in range(NG):
                        nc.tensor.matmul(
                            pv2[:, n * 512:(n + 1) * 512],
                            XT[:, 2 * g:2 * g + 2, tt * SUB:(tt + 1) * SUB],
                            wv_sb[:, 2 * g:2 * g + 2, n * 512:(n + 1) * 512],
                            start=(g == 0), stop=(g == NG - 1), perf_mode=DR)
                nc.scalar.activation(vt[:], pv2[:], AF.Identity,
                                     scale=SA / (SX * SW))
                v_tiles.append(vt)

            if stop_stage == 'v':
                dbg = yp.tile([128, D_MODEL], dt.float32, tag="y")
                nc.vector.tensor_copy(dbg[:], v_tiles[0][:])
                nc.sync.dma_start(ys[t0:t0 + SUB, :], dbg[:])
                continue

            # ---------- rope: shuffle (+-32 partitions) and combine --------
            # per et-half for pipelining: Pool does the tan multiply,
            # DVE the add (Pool cannot touch PSUM so it gets SBUF work)
            tan_bc_half = tan_sb[:].unsqueeze(1).to_broadcast(
                (128, (ND // 2) * BLK // CHUNK, CHUNK))
            for src_t, deng in ((q_all, nc.sync), (k_all, nc.scalar)):
                for hf in range(2):
                    sl = slice(4 * hf, 4 * hf + 4)
                    qs = qsp.tile([128, ND // 2, BLK], dt.float16, tag="qs")
                    for (o, i) in ((0, 32), (32, 0), (64, 96), (96, 64)):
                        deng.dma_start(qs[o:o + 32, :, :], src_t[i:i + 32, sl, :])
                    nc.gpsimd.tensor_tensor(
                        qs[:].rearrange("p a (b c) -> p (a b) c", c=CHUNK),
                        qs[:].rearrange("p a (b c) -> p (a b) c", c=CHUNK),
                        tan_bc_half, op=OP.mult)
                    nc.vector.tensor_tensor(src_t[:, sl, :], src_t[:, sl, :],
                                            qs[:], op=OP.add)

            if it + 1 < NBLK * repeats:
                staged_xt = convert_stage(staged[2])

            if stop_stage == 'rope':
                dbg = yp.tile([128, D_MODEL], dt.float32, tag="y")
                nc.vector.tensor_copy(dbg[:, 0:512], q_all[:, 0, :])
                nc.sync.dma_start(ys[t0:t0 + SUB, :], dbg[:])
                continue

            # ---------- attention per 128-token tile ----------
            for tt in range(NSUB):
                pexp_tiles = []
                bidx = (t0 // SUB) + tt
                for hg in range(4):
                    heads = [(hg // 2) * 8 + (hg % 2) + 2 * hh for hh in range(4)]
                    sps = psA.tile([128, BLK], dt.float32, tag="ps512")
                    # seed the block-diagonal mask bias: rank-5 const matmul
                    nc.tensor.matmul(sps[:], c5_sb[0:5, :], brow5_sb[0:5, :],
                                     start=True, stop=False,
                                     skip_group_check=True)
                    for hh, h in enumerate(heads):
                        et, po = h // 2, (h % 2) * 64
                        ksl = k_all[po:po + 64, et, tt * SUB:(tt + 1) * SUB]
                        qsl = q_all[po:po + 64, et, tt * SUB:(tt + 1) * SUB]
                        nc.tensor.matmul(sps[:, hh * 128:(hh + 1) * 128],
                                         ksl, qsl, start=False, stop=True,
                                         skip_group_check=True)
                    pexp = pep.tile([128, BLK], dt.float16, tag="pe")
                    nc.scalar.activation(pexp[:], sps[:], AF.Exp,
                                         scale=float(D_HEAD) ** -0.5 / (SX * SW) ** 2,
                                         bias=kb_sb[:, bidx:bidx + 1])
                    pexp_tiles.append(pexp)
                    # column sums stacked at partition base 32*hg of the
                    # shared sums bank (one reciprocal per tile)
                    nc.tensor.matmul(sums_bank[32 * hg:32 * hg + 1, :],
                                     ones_sb[:], pexp[:],
                                     start=True, stop=True,
                                     tile_position=(0, 32 * hg))

                if stop_stage in ('attn', 'attn1'):
                    dbg = yp.tile([128, D_MODEL], dt.float32, tag="y")
                    nc.vector.tensor_copy(dbg[:, 0:512], pexp_tiles[0][:])
                    nc.sync.dma_start(ys[t0 + tt * SUB:t0 + (tt + 1) * SUB, :], dbg[:])
                    continue

                rc = rcp.tile([128, BLK], dt.float16, tag="rc")
                with nc.allow_low_precision(reason="softmax denominators are O(1..1e4); fp16 recip is plenty"):
                    nc.vector.reciprocal(rc[:], sums_bank[:])

                # broadcast 1/sums to a [128,1024] psum tile laid out to match
                # A^T: rows 0:64 even heads (dp = head//2), rows 64:128 odd
                recipT = psB.tile([128, D_MODEL], dt.float32, tag="big")
                for hg in range(4):
                    rowb = 64 * (hg % 2)
                    colb = 512 * (hg // 2)
                    nc.tensor.matmul(recipT[rowb:rowb + 64, colb:colb + 512],
                                     ones64_sb[32 * hg:32 * hg + 1, :],
                                     rc[32 * hg:32 * hg + 1, :],
                                     start=True, stop=True,
                                     tile_position=(32 * hg, rowb))

                rts = rcp.tile([128, D_MODEL], dt.float16, tag="rts")
                nc.scalar.copy(rts[:], recipT[:])

                # ---------- PV on masked pexp; normalize at eviction -------
                ap_t = psB.tile([128, D_MODEL], dt.float32, tag="big")
                for h in range(N_HEADS):
                    g = 2 * (h // 8) + (h % 2)
                    col = (h % 8) // 2
                    po = (h % 2) * 64
                    dp = h // 2
                    nc.tensor.matmul(
                        ap_t[po:po + 64, dp * 128:(dp + 1) * 128],
                        v_tiles[tt][:, h * D_HEAD:(h + 1) * D_HEAD],
                        pexp_tiles[g][:, col * 128:(col + 1) * 128],
                        start=True, stop=True, tile_position=(0, po))
                asb = asp.tile([128, ND, SUB], dt.float8e4, tag="a")
                nc.vector.tensor_tensor(
                    asb[:],
                    ap_t[:].rearrange("p (a c) -> p a c", c=SUB),
                    rts[:].rearrange("p (a c) -> p a c", c=SUB),
                    op=OP.mult)

                if stop_stage == 'pv':
                    dbg = yp.tile([128, D_MODEL], dt.float32, tag="y")
                    nc.vector.tensor_copy(dbg[:], asb[:].rearrange("p a c -> p (a c)"))
                    nc.sync.dma_start(ys[t0 + tt * SUB:t0 + (tt + 1) * SUB, :], dbg[:])
                    continue

                # ---------- out projection (fp8 DoubleRow) + residual ------
                ops = psB.tile([128, D_MODEL], dt.float32, tag="big")
                for n in range(2):
                    for i in range(4):
                        nc.tensor.matmul(
                            ops[:, n * 512:(n + 1) * 512],
                            asb[:, 2 * i:2 * i + 2, :],
                            wo_sb[:, 2 * i:2 * i + 2, n * 512:(n + 1) * 512],
                            start=(i == 0), stop=(i == 3), perf_mode=DR)
                rows = slice(t0 + tt * SUB, t0 + (tt + 1) * SUB)
                y = yp.tile([128, D_MODEL], dt.float32, tag="y")
                nc.vector.scalar_tensor_tensor(y[:], ops[:], 1.0 / (SA * SW2),
                                               x_tiles[tt][:], op0=OP.mult,
                                               op1=OP.add)
                nc.gpsimd.dma_start(ys[rows, :], y[:])

    nc.compile()
    return nc


def host_inputs(x, mask, ln_gamma, ln_beta, W_qkv, W_out, T):
    """Prepare per-core input maps. x: (B, L, D) fp32."""
    import ml_dtypes
    f8 = ml_dtypes.float8_e4m3

    B, L, D = x.shape
    tokens = B * L
    n_cores = tokens // T
    W_eff = (W_qkv * ln_gamma[None, :]).astype(np.float32)

    # qk weights: [128, ND, 2D]; wqk8[p, i, o] = W^T[128i + p, o]
    WqkT = np.ascontiguousarray(W_eff[0:2 * D].T) * SW       # (D, 2D)
    wqk8 = WqkT.reshape(ND, 128, 2 * D).transpose(1, 0, 2)
    wqk8 = np.ascontiguousarray(wqk8).astype(f8)
    WvT = np.ascontiguousarray(W_eff[2 * D:3 * D].T) * SW    # (D, D)
    wv8 = WvT.reshape(ND, 128, D).transpose(1, 0, 2)
    wv8 = np.ascontiguousarray(wv8).astype(f8)
    # out-proj: wo8[p, i, o] = W_out^T[128i + p, o]
    WoT = np.ascontiguousarray(W_out.T) * SW2                # (D, D)
    wo8 = WoT.reshape(ND, 128, D).transpose(1, 0, 2)
    wo8 = np.ascontiguousarray(wo8).astype(f8)

    inv_freq = 1.0 / (ROPE_BASE ** (np.arange(0, D_HEAD, 2) / D_HEAD))  # (32,)
    p = np.arange(128)
    j = p % D_HEAD
    idx = j % 32
    sign = np.where(j < 32, -1.0, 1.0)
    t = np.arange(CHUNK)
    ang = t[None, :] * inv_freq[idx][:, None]          # (128, 32)
    cos_h = np.cos(ang).astype(np.float16)
    tan_h = (sign[:, None] * np.tan(ang)).astype(np.float16)

    # rank-5 mask bias: B[p, q] = BIG * [chunk(p) == chunk(q)] - BIG, where
    # BIG is pre-scaled so exp's dequant scale maps it back to MASKB
    big = MASKB * (SX * SW) ** 2
    c5_h = np.zeros((128, 128), dtype=ml_dtypes.bfloat16)
    for c in range(4):
        c5_h[c, c * 32:(c + 1) * 32] = 1.0   # row c: indicator of chunk c (cols = p)
    c5_h[4, :] = 1.0
    brow5_h = np.zeros((128, BLK), dtype=np.float32)
    qpos = np.arange(BLK)
    qc = (qpos % 128) // 32
    for c in range(4):
        brow5_h[c, :] = np.where(qc == c, big, 0.0)
    brow5_h[4, :] = -big
    brow5_h = brow5_h.astype(ml_dtypes.bfloat16)

    xs_flat = np.ascontiguousarray(x.reshape(tokens, D).astype(np.float32))
    mask_flat = mask.reshape(tokens).astype(np.float32)
    kbias = np.where(mask_flat == 0, -MASKB, 0.0).astype(np.float32)

    shared = {"wqk": wqk8, "wv": wv8, "wo": wo8,
              "cosb": cos_h, "tanb": tan_h, "c5": c5_h, "brow5": brow5_h}
    with_beta = bool(np.any(ln_beta != 0))
    if with_beta:
        shared["beta"] = np.tile((ln_beta * SX).astype(np.float16)[None, :],
                                 (128, 1))

    in_maps = []
    for c in range(n_cores):
        sl = slice(c * T, (c + 1) * T)
        kb_c = np.ascontiguousarray(
            kbias[sl].reshape(T // 128, 128).T).astype(np.float32)
        m = dict(shared)
        m["xs"] = xs_flat[sl]
        m["kb"] = kb_c
        in_maps.append(m)
    return in_maps, with_beta


_PROGRAM_CACHE = {}


def kernel(x, mask, ln_gamma, ln_beta, W_qkv, W_out):
    from concourse import bass_utils

    x = np.asarray(x, dtype=np.float32)
    mask = np.asarray(mask, dtype=np.float32)
    ln_gamma = np.asarray(ln_gamma, dtype=np.float32)
    ln_beta = np.asarray(ln_beta, dtype=np.float32)
    W_qkv = np.asarray(W_qkv, dtype=np.float32)
    W_out = np.asarray(W_out, dtype=np.float32)

    B, L, D = x.shape
    T = (B * L) // N_CORES
    in_maps, with_beta = host_inputs(x, mask, ln_gamma, ln_beta, W_qkv, W_out, T)

    key = (T, with_beta)
    if key not in _PROGRAM_CACHE:
        _PROGRAM_CACHE[key] = build_program(T, with_beta=with_beta)
    nc = _PROGRAM_CACHE[key]

    res = bass_utils.run_bass_kernel_spmd(nc, in_maps, core_ids=list(range(N_CORES)))
    ys = np.concatenate([res.results[c]["ys"] for c in range(N_CORES)], axis=0)
    return ys.reshape(B, L, D).astype(np.float32)


if __name__ == "__main__":
    rng = np.random.default_rng(0)
    B, L = 4, 4096
    x = rng.standard_normal((B, L, D_MODEL), dtype=np.float32)
    mask = np.ones((B, L), dtype=np.float32)
    g = np.ones(D_MODEL, dtype=np.float32)
    be = np.zeros(D_MODEL, dtype=np.float32)
    Wq = (rng.standard_normal((3 * D_MODEL, D_MODEL)) * 0.02).astype(np.float32)
    Wo = (rng.standard_normal((D_MODEL, D_MODEL)) * 0.02).astype(np.float32)
    y = kernel(x, mask, g, be, Wq, Wo)
    print("kernel output:", y.shape, y.dtype)
